# revision 50
# baseline (speedup 1.0000x reference)
"""Two-layer GCN (AttributeDecoder) as a distributed Bass kernel on 8 TRN2 NeuronCores.

Math (per reference):
    dis = (deg of A+I)^-1/2
    L1:  relu1 = relu( D @ ((A+I) @ (D @ x)) @ W1 + b1 )   with D = diag(dis)
    L2:  out   = relu( D @ ((A+I) @ (D @ relu1)) @ W2 + b2 )
using (A_hat @ h) @ W == A_hat @ (h @ W) so both layers aggregate 64-wide
features before the dense W matmul.

Sharding: destination nodes (and their in-edges) are partitioned contiguously
across the 8 cores; within a core, nodes are re-assigned to 128-node blocks by
a greedy balance of per-block in-edge counts (per source parity), which
minimizes the uniform subtile count T2.

Per destination block of 128 nodes, edges are processed in subtiles of 128
slots: a message tile [128 edges, 64 feats] is combined with a host-built
selection matrix S' (S'[e, n] = dis[src(e)] iff dst(e) == n else 0; the last
subtile is diag(dis) for the self loops) via TensorE matmuls accumulating in
PSUM, then the dense W matmul, dis[dst] scaling, bias and relu per block.
The S' images are static (graph structure) and streamed from HBM, so the
vector engine only runs the small per-block epilogues.

Layer 1 messages need no on-device gather at all: the host materializes the
edge-slot-ordered stream of source features (bf16 copy of x rows) which the
kernel streams sequentially at line rate.  Layer 2 messages depend on layer-1
output, so they are fetched with dma_gather (int16 indices; even/odd source
parity fetched at 256B pair stride) from the AllGathered relu1 table, with
descriptor generation spread over 4 SWDGE queues (all 4 Q7 core pairs).
The relu1 table is exchanged with chunked AllGathers that overlap the tail of
layer-1 compute.
"""

import numpy as np
import ml_dtypes

from concourse import bass, mybir, bacc
import concourse.tile as tile
from concourse.bass_utils import run_bass_kernel_spmd

BF16 = ml_dtypes.bfloat16
P = 128
N_CORES = 8
G = 4               # dst blocks per gather/stream group
NQ = 4              # SWDGE queues (all 4 Q7 core pairs generate concurrently)
CSL = 16            # slots per gather call


def _balance_blocks(dE, dO, par_n, nb, target):
    """Assign nodes to blocks (64 even-id + 64 odd-id slots each) greedily
    minimizing the max per-parity edge load, then refine toward `target`
    max load per (block, parity). Returns (block, pos) per node."""
    nsh = len(dE)
    loadE = np.zeros(nb, dtype=np.int64)
    loadO = np.zeros(nb, dtype=np.int64)
    cnt = np.zeros((nb, 2), dtype=np.int64)     # slots used per id-parity
    block = np.zeros(nsh, dtype=np.int64)
    order = np.argsort(-(dE + dO), kind="stable")
    for n in order:
        q = par_n[n]
        cand = np.where(cnt[:, q] < P // 2)[0]
        scores = np.maximum(loadE[cand] + dE[n], loadO[cand] + dO[n])
        b = cand[np.argmin(scores)]
        block[n] = b
        loadE[b] += dE[n]
        loadO[b] += dO[n]
        cnt[b, q] += 1
    # refinement: move nodes out of (block, parity) bins above target
    loads = [loadE, loadO]
    degs = [dE, dO]
    for _ in range(6000):
        hot_par = 0 if loadE.max() >= loadO.max() else 1
        hot = int(np.argmax(loads[hot_par]))
        over = loads[hot_par][hot] - target
        if over <= 0:
            break
        members = np.where(block == hot)[0]
        dh = degs[hot_par][members]
        cand_n = members[np.argsort(-np.minimum(dh, over))[:6]]
        best = None
        for n in cand_n:
            q = par_n[n]
            ok = cnt[:, q] < P // 2
            ok[hot] = False
            if not ok.any():
                continue
            newmax = np.maximum(loadE + dE[n], loadO + dO[n])
            newmax[~ok] = 1 << 60
            b2 = int(np.argmin(newmax))
            peak = max(newmax[b2],
                       loadE[hot] - dE[n], loadO[hot] - dO[n])
            if best is None or peak < best[0]:
                best = (peak, n, b2)
        if best is None:
            break
        cur = max(loadE.max(), loadO.max())
        peak, n, b2 = best
        if peak > cur:
            break
        q = par_n[n]
        block[n] = b2
        loadE[hot] -= dE[n]; loadO[hot] -= dO[n]
        loadE[b2] += dE[n]; loadO[b2] += dO[n]
        cnt[hot, q] -= 1; cnt[b2, q] += 1
    # positions: even-id nodes at even positions, odd at odd (keeps the
    # layer-2 table row parity equal to the node id parity)
    pos = np.zeros(nsh, dtype=np.int64)
    ctr = np.zeros((nb, 2), dtype=np.int64)
    for n in range(nsh):
        b, q = block[n], par_n[n]
        pos[n] = 2 * ctr[b, q] + q
        ctr[b, q] += 1
    return block, pos


def _preprocess(x, edge_index, W1, b1, W2, b2):
    n = x.shape[0]
    f1 = x.shape[1]
    f2 = W2.shape[1]
    assert n % N_CORES == 0
    nsh = n // N_CORES
    assert nsh % 2 == 0

    ei = np.asarray(edge_index).astype(np.int64)
    src = ei[0].copy()
    dst = ei[1].copy()

    deg = np.bincount(dst, minlength=n).astype(np.float32) + 1.0  # + self loop
    dis = (1.0 / np.sqrt(deg)).astype(np.float32)

    owner = dst // nsh
    par = (src % 2).astype(np.int64)

    # pick the block count minimizing total slot count nb*2*T2 (an extra
    # block can lower the per-(block,parity) ceiling T2)
    pmax = 0
    for c in range(N_CORES):
        m = owner == c
        pmax = max(pmax, int((par[m] == 0).sum()), int((par[m] == 1).sum()))
    nbmin = (nsh + P - 1) // P
    best_nb, best_slots = None, None
    for nb_c in (nbmin, nbmin + 1, nbmin + 2):
        # need enough id-parity slots per core
        if nb_c * (P // 2) < (nsh + 1) // 2:
            continue
        t2lb = max(1, -(-pmax // (nb_c * P)))
        slots = nb_c * 2 * t2lb
        if best_slots is None or slots < best_slots:
            best_nb, best_slots = nb_c, slots
    nb = best_nb
    nsh_pad = nb * P
    t2_goal = max(1, -(-pmax // (nb * P)))

    # chunked AllGather: small first chunk (absorbs collective warmup),
    # small last chunk (low tail exposure)
    if nb >= 16:
        nch = 6
        cb = [0, max(1, nb // 25), max(2, 9 * nb // 50), 19 * nb // 50,
              31 * nb // 50, 44 * nb // 50, nb]
    elif nb >= 10:
        nch = 4
        cb = [0, nb // 8, nb // 2, nb - max(1, nb // 5), nb]
    elif nb >= 6:
        nch = 3
        cb = [0, max(1, nb // 6), nb - max(1, nb // 5), nb]
    else:
        nch = min(2, nb)
        cb = [(k * nb) // nch for k in range(nch + 1)]
    csz = [(cb[k + 1] - cb[k]) * P for k in range(nch)]  # rows per core/chunk
    off = np.zeros(nch + 1, dtype=np.int64)
    for k in range(nch):
        off[k + 1] = off[k] + N_CORES * csz[k]

    # per-core balanced node->(block, pos) assignment
    blocks_all = np.zeros(n, dtype=np.int64)
    pos_all = np.zeros(n, dtype=np.int64)
    for c in range(N_CORES):
        lo, hi = c * nsh, (c + 1) * nsh
        m = (dst >= lo) & (dst < hi)
        dloc = dst[m] - lo
        dE = np.bincount(dloc[par[m] == 0], minlength=nsh)
        dO = np.bincount(dloc[par[m] == 1], minlength=nsh)
        par_n = np.arange(nsh) % 2
        blk, pos = _balance_blocks(dE, dO, par_n, nb, t2_goal * P)
        blocks_all[lo:hi] = blk
        pos_all[lo:hi] = pos

    # layer-2 table row for each global node (chunk-major AllGather layout)
    cb_a = np.asarray(cb)
    csz_a = np.asarray(csz)
    chunk_of = np.searchsorted(cb_a, blocks_all, side="right") - 1
    row2_all = (
        off[chunk_of]
        + (np.arange(n) // nsh) * csz_a[chunk_of]
        + (blocks_all - cb_a[chunk_of]) * P
        + pos_all
    )
    # row parity must equal node-id parity (for the shared parity split)
    assert ((row2_all % 2) == (np.arange(n) % 2)).all()

    # per-(core, block, parity) counts -> uniform external subtile count T2
    e_blk = blocks_all[dst]
    cnt = np.zeros((N_CORES, nb, 2), dtype=np.int64)
    np.add.at(cnt, (owner, e_blk, par), 1)
    T2 = max(1, int((cnt.max() + P - 1) // P))
    TS = 2 * T2                       # external subtile slots per block
    TT = TS + 1                       # + self subtile

    n_groups = (nb + G - 1) // G

    def wrap16(flat):
        cols = len(flat) // 16
        img = flat.reshape(cols, 16).T
        return np.tile(img, (8, 1)).astype(np.int16)

    xbf = np.asarray(x, dtype=np.float32).astype(BF16)

    in_maps = []
    for c in range(N_CORES):
        lo = c * nsh
        m = owner == c
        s_c = src[m]
        b_c = e_blk[m]
        p_c = pos_all[dst[m]]
        g_c = b_c * 2 + par[m]
        d_c = dis[s_c]                      # dis[src] per edge

        # within each (block, parity) run, order edges by table row so the
        # gather's 256B fetches walk the table in ascending address order
        # (better HBM locality during SDMA drain); the aggregation is a sum,
        # so slot order within a run is free
        order = np.lexsort((row2_all[s_c], g_c))
        s_c, b_c, p_c, g_c, d_c = (
            s_c[order], b_c[order], p_c[order], g_c[order], d_c[order])
        cnt_c = cnt[c].reshape(-1)
        start = np.zeros(nb * 2, dtype=np.int64)
        start[1:] = np.cumsum(cnt_c)[:-1]
        slot = np.arange(len(g_c)) - start[g_c]
        par_c = g_c % 2
        tsub = slot // P                      # subtile within parity run
        lane = slot % P                       # partition lane
        tp = par_c * T2 + tsub                # subtile index in [0, TS)

        # --- layer-2 gather indices (pair rows of the AllGathered table),
        # laid out in gather-call order (parity-major within each group) ---
        lin = g_c * (T2 * P) + slot
        src2h = np.zeros(nb * TS * P, dtype=np.int64)
        src2h[lin] = row2_all[s_c] >> 1

        src2h_r = src2h.reshape(nb, TS, P)

        def call_order(a):
            segs = []
            for g in range(n_groups):
                g0, g1 = g * G, min(g * G + G, nb)
                segs.append(a[g0:g1, :T2].reshape(-1, P))
                segs.append(a[g0:g1, T2:].reshape(-1, P))
            return np.concatenate(segs).reshape(-1)

        src2_img = wrap16(call_order(src2h_r))

        # --- S' images: S'[lane, b, t, col] = dis[src] iff dst(edge) == col;
        # self subtile t == TS carries diag(dis) ---
        simg = np.zeros((P, nb, TT, P), dtype=BF16)
        simg[lane, b_c, tp, p_c] = d_c.astype(BF16)

        # --- layer-1 message stream: x[src] per slot, b-major x t layout ---
        m1 = np.zeros((P, nb, TS, f1), dtype=BF16)
        m1[lane, b_c, tp, :] = xbf[s_c]

        # own nodes in (block, pos) layout
        node_at = np.full(nsh_pad, -1, dtype=np.int64)
        node_at[blocks_all[lo : lo + nsh] * P + pos_all[lo : lo + nsh]] = (
            np.arange(nsh)
        )
        occ = node_at >= 0
        xo = np.zeros((nsh_pad, f1), dtype=BF16)
        xo[occ] = xbf[lo + node_at[occ]]
        dv = np.zeros(nsh_pad, dtype=np.float32)
        dv[occ] = dis[lo + node_at[occ]]
        dis_col = dv.reshape(nb, P).T.copy()

        # self-loop diag into S'
        pp = np.arange(nsh_pad)
        simg[pp % P, pp // P, TS, pp % P] = dv.astype(BF16)

        in_maps.append(
            {"src2": src2_img,
             "sp": simg.reshape(P, nb * TT * P),
             "m1": m1.reshape(P, nb * TS * f1),
             "dis_col": dis_col, "xon": xo, "node_at": node_at}
        )

    shared = {
        "w1": np.asarray(W1, dtype=np.float32).astype(BF16),
        "w2": np.asarray(W2, dtype=np.float32).astype(BF16),
        "b1b": np.tile(np.asarray(b1, dtype=np.float32), (P, 1)),
        "b2b": np.tile(np.asarray(b2, dtype=np.float32), (P, 1)),
    }
    for m_ in in_maps:
        m_.update(shared)

    cfg = dict(n=n, f1=f1, f2=f2, nsh=nsh, nb=nb, nsh_pad=nsh_pad, T2=T2,
               TS=TS, TT=TT, n_groups=n_groups,
               nch=nch, cb=cb, csz=csz, off=off.tolist())
    return in_maps, cfg


def _pairs_ap(handle, n_rows, f1):
    """view table [n_rows, f1] as items of row PAIRS: item k -> rows (2k, 2k+1)"""
    ap = handle.ap()
    return bass.AP(ap.tensor, 0, [[2 * f1, n_rows // 2], [1, 2 * f1]])


def _build(cfg):
    nb, T2, TS, TT = (cfg[k] for k in ("nb", "T2", "TS", "TT"))
    f1, f2, nsh_pad, n_groups = (
        cfg[k] for k in ("f1", "f2", "nsh_pad", "n_groups"))
    nch, cb, csz, off = (cfg[k] for k in ("nch", "cb", "csz", "off"))
    dt = mybir.dt
    idx_cols = nb * TS * P // 16

    nc = bacc.Bacc("TRN2", target_bir_lowering=False, debug=False,
                   num_devices=N_CORES, num_swdge_queues=NQ)

    xon = nc.dram_tensor("xon", [nsh_pad, f1], dt.bfloat16, kind="ExternalInput")
    w1 = nc.dram_tensor("w1", [f1, f1], dt.bfloat16, kind="ExternalInput")
    w2 = nc.dram_tensor("w2", [f1, f2], dt.bfloat16, kind="ExternalInput")
    b1b = nc.dram_tensor("b1b", [P, f1], dt.float32, kind="ExternalInput")
    b2b = nc.dram_tensor("b2b", [P, f2], dt.float32, kind="ExternalInput")
    src2 = nc.dram_tensor("src2", [P, idx_cols], dt.int16, kind="ExternalInput")
    sp = nc.dram_tensor("sp", [P, nb * TT * P], dt.bfloat16,
                        kind="ExternalInput")
    m1 = nc.dram_tensor("m1", [P, nb * TS * f1], dt.bfloat16,
                        kind="ExternalInput")
    dis_col = nc.dram_tensor("dis_col", [P, nb], dt.float32, kind="ExternalInput")
    out = nc.dram_tensor("out", [nsh_pad, f2], dt.float32, kind="ExternalOutput")

    r1s_own = nc.dram_tensor("r1s_own", [nsh_pad, f1], dt.bfloat16)
    r1s_full = nc.dram_tensor("r1s_full", [N_CORES * nsh_pad, f1], dt.bfloat16,
                              addr_space="Shared")
    cc_warm_in = nc.dram_tensor("cc_warm_in", [1, P], dt.float32)
    cc_warm_out = nc.dram_tensor("cc_warm_out", [N_CORES, P], dt.float32,
                                 addr_space="Shared")

    sp_ap = sp.ap()
    m1_ap = m1.ap()

    with tile.TileContext(nc) as tc:
        with (
            tc.tile_pool(name="const", bufs=1) as constp,
            tc.tile_pool(name="msg", bufs=18) as msgp,
            tc.tile_pool(name="m1l", bufs=4) as m1p,
            tc.tile_pool(name="smat", bufs=6) as smatp,
            tc.tile_pool(name="eplg", bufs=6) as eplgp,
            tc.tile_pool(name="acc", bufs=1) as accp,
            tc.tile_pool(name="outg", bufs=2) as outgp,
            tc.tile_pool(name="ps1", bufs=4, space="PSUM") as ps1p,
            tc.tile_pool(name="ps2", bufs=4, space="PSUM") as ps2p,
        ):
            # ---- constants ----
            w1_sb = constp.tile([f1, f1], dt.bfloat16)
            nc.sync.dma_start(out=w1_sb[:], in_=w1.ap())
            w2_sb = constp.tile([f1, f2], dt.bfloat16)
            nc.sync.dma_start(out=w2_sb[:], in_=w2.ap())
            b1_sb = constp.tile([P, f1], dt.float32)
            nc.sync.dma_start(out=b1_sb[:], in_=b1b.ap())
            b2_sb = constp.tile([P, f2], dt.float32)
            nc.sync.dma_start(out=b2_sb[:], in_=b2b.ap())
            dis_col_sb = constp.tile([P, nb], dt.float32)
            nc.sync.dma_start(out=dis_col_sb[:], in_=dis_col.ap())
            src2_sb = constp.tile([P, idx_cols], dt.int16)
            nc.scalar.dma_start(out=src2_sb[:], in_=src2.ap())
            xon_sb = constp.tile([P, nb, f1], dt.bfloat16)
            nc.scalar.dma_start(out=xon_sb[:],
                                in_=xon.ap().rearrange("(b p) f -> p b f", p=P))

            qctr = [0]
            PF = 3              # gather prefetch depth (groups)

            def layer(is_l1, selftab, w_sb, b_sb, fo, emit):
                gmeta = []
                sb = 0
                for g in range(n_groups):
                    g0, g1 = g * G, min(g * G + G, nb)
                    gmeta.append((g0, g1, g1 - g0, (g1 - g0) * T2, sb))
                    sb += (g1 - g0) * TS
                gather_tiles = {}

                def issue_gathers(g):
                    g0, g1, gb, half, slot_base = gmeta[g]
                    tiles = []
                    for s0 in range(0, 2 * half, CSL):
                        s1 = min(s0 + CSL, 2 * half)
                        i0 = (slot_base + s0) * P
                        n_idx = (s1 - s0) * P
                        mcall = msgp.tile([P, CSL, 2 * f1], dt.bfloat16,
                                          tag="msg")
                        tiles.append(mcall)
                        nc.gpsimd.dma_gather(
                            out_ap=mcall[:, : s1 - s0, :],
                            in_ap=_pairs_ap(r1s_full, N_CORES * nsh_pad, f1),
                            idxs_ap=src2_sb[:, i0 // 16 : (i0 + n_idx) // 16],
                            num_idxs=n_idx,
                            num_idxs_reg=n_idx,
                            elem_size=2 * f1,
                            elem_step=2 * f1,
                            single_packet=False,
                            queue_num=qctr[0] % NQ,
                        )
                        qctr[0] += 1
                    gather_tiles[g] = tiles

                if not is_l1:
                    for g in range(min(PF, n_groups)):
                        issue_gathers(g)
                for g in range(n_groups):
                    g0, g1, gb, half, slot_base = gmeta[g]
                    # S' images per block pair, alternating HWDGE rings
                    sgs = []
                    mts = []
                    for q in range((gb + 1) // 2):
                        b0 = g0 + 2 * q
                        b1 = min(b0 + 2, g1)
                        eng = nc.scalar if q % 2 == 0 else nc.sync
                        sgq = smatp.tile([P, 2 * TT, P], dt.bfloat16,
                                         tag="smat")
                        eng.dma_start(
                            out=sgq[:, : (b1 - b0) * TT, :],
                            in_=sp_ap[:, b0 * TT * P : b1 * TT * P])
                        sgs.append(sgq)
                        if is_l1:
                            eng2 = nc.sync if q % 2 == 0 else nc.scalar
                            mtq = m1p.tile([P, 2 * TS, f1], dt.bfloat16,
                                           tag="m1t")
                            eng2.dma_start(
                                out=mtq[:, : (b1 - b0) * TS, :],
                                in_=m1_ap[:, b0 * TS * f1 : b1 * TS * f1])
                            mts.append(mtq)
                    if not is_l1:
                        if g + PF < n_groups:
                            issue_gathers(g + PF)
                        call_tiles = gather_tiles.pop(g)
                    for j, b in enumerate(range(g0, g1)):
                        ps1 = ps1p.tile([f1, P], dt.float32, space="PSUM",
                                        tag="ps1")
                        for t in range(TT):
                            if t < TS:
                                if is_l1:
                                    lhsT = mts[j // 2][:, (j % 2) * TS + t, :]
                                else:
                                    parity, tsub = (
                                        (0, t) if t < T2 else (1, t - T2))
                                    sgrp = parity * half + j * T2 + tsub
                                    lhsT = call_tiles[sgrp // CSL][
                                        :, sgrp % CSL,
                                        parity * f1 : parity * f1 + f1]
                            else:
                                lhsT = selftab[:, b, :f1]
                            nc.tensor.matmul(
                                out=ps1[:],
                                lhsT=lhsT,
                                rhs=sgs[j // 2][:, (j % 2) * TT + t, :],
                                start=(t == 0),
                                stop=(t == TT - 1),
                            )
                        aggT = eplgp.tile([f1, P], dt.bfloat16, tag="aggT")
                        nc.vector.tensor_copy(aggT[:], ps1[:])
                        ps2 = ps2p.tile([P, fo], dt.float32, space="PSUM",
                                        tag="ps2")
                        nc.tensor.matmul(out=ps2[:], lhsT=aggT[:], rhs=w_sb[:],
                                         start=True, stop=True)
                        tt = eplgp.tile([P, fo], dt.float32, tag="tt")
                        nc.vector.scalar_tensor_tensor(
                            out=tt[:],
                            in0=ps2[:],
                            scalar=dis_col_sb[:, b : b + 1],
                            in1=b_sb[:],
                            op0=mybir.AluOpType.mult,
                            op1=mybir.AluOpType.add,
                        )
                        emit(b, tt)
                    slot_base += gb * TS

            # ---- L1 ----
            r1s_sb = accp.tile([P, nb, f1], dt.bfloat16)
            r1s_own_r = r1s_own.ap().rearrange("(b p) f -> p b f", p=P)
            next_chunk = [0]

            def emit1(b, tt):
                nc.vector.tensor_scalar_max(r1s_sb[:, b, :], tt[:], 0.0)
                k = next_chunk[0]
                if k < nch and b == cb[k + 1] - 1:
                    nc.sync.dma_start(out=r1s_own_r[:, cb[k] : cb[k + 1], :],
                                      in_=r1s_sb[:, cb[k] : cb[k + 1], :])
                    nc.gpsimd.collective_compute(
                        "AllGather",
                        mybir.AluOpType.bypass,
                        replica_groups=[list(range(N_CORES))],
                        ins=[r1s_own.ap()[cb[k] * P : cb[k + 1] * P, :].opt()],
                        outs=[r1s_full.ap()[off[k] : off[k + 1], :].opt()],
                    )
                    next_chunk[0] += 1

            layer(True, xon_sb, w1_sb, b1_sb, f1, emit1)

            # ---- L2 ----
            out_r = out.ap().rearrange("(b p) f -> p b f", p=P)
            og_cur = {}

            def emit2(b, tt):
                if b % G == 0:
                    ogt = outgp.tile([P, G, f2], dt.float32, tag="og")
                    og_cur["t"] = ogt
                    og_cur["b0"] = b
                og, b0 = og_cur["t"], og_cur["b0"]
                nc.vector.tensor_scalar_max(og[:, b - b0, :], tt[:], 0.0)
                if b - b0 == G - 1 or b == nb - 1:
                    nc.sync.dma_start(out=out_r[:, b0 : b + 1, :],
                                      in_=og[:, : b - b0 + 1, :])

            layer(False, r1s_sb, w2_sb, b2_sb, f2, emit2)

    nc.compile()
    return nc


_CACHE = {}


def kernel(x, edge_index, W1, b1, W2, b2, _want_profile=False):
    x = np.asarray(x)
    in_maps, cfg = _preprocess(x, edge_index, W1, b1, W2, b2)
    key = (cfg["n"], cfg["f1"], cfg["f2"], cfg["T2"])
    if key not in _CACHE:
        _CACHE[key] = _build(cfg)
    nc = _CACHE[key]
    node_ats = [m.pop("node_at") for m in in_maps]
    res = run_bass_kernel_spmd(
        nc, in_maps, core_ids=list(range(N_CORES)), trace=_want_profile
    )
    nsh = cfg["nsh"]
    full = np.empty((cfg["n"], cfg["f2"]), dtype=np.float32)
    for c in range(N_CORES):
        o = res.results[c]["out"]
        na = node_ats[c]
        occ = na >= 0
        full[c * nsh + na[occ]] = o[occ]
    if _want_profile:
        return full, res
    return full
